# revision 10
# baseline (speedup 1.0000x reference)
"""ListNet T1 loss on 8 Trainium2 NeuronCores.

Math: for each ragged segment s (contiguous slice of the flat arrays),
    loss_s = log(sum_i exp(m_i)) - (sum_i exp(t_i)*m_i) / (sum_i exp(t_i))
    out    = sum_s loss_s / S
(softmax / log-softmax are shift invariant; inputs are ~N(0,1) so exp() is
safe in fp32 without the max-subtraction trick).

Per core (2048 consecutive segments): sort segments by length, place them as
rows of [128, K*F] SBUF tiles via indirect (gather) DMA with element-granular
offsets, mask the padded row tails to -1e30, then:
  A = sum exp(t)  via ACT Exp + accum_out
  C = sum exp(m)  via ACT Exp + accum_out
  B = sum exp(t)*m via DVE tensor_tensor_reduce
Combine per segment, reduce on-chip to one scalar per core, sum on host.

The final global segment is an outlier (~13.5k elements, vs <=896 for all
others): it is excluded from the sorted grid (its slot becomes a length-1
dummy at offset 0, whose loss is ln(exp(m0)) - m0 ~= 0) and processed by a
dedicated block: one contiguous DMA reshaped to [128, W8], masked, with the
partial A/B/C summed across partitions (ones-matmul) before the log/ratio.
Every core runs that block; non-owner cores get a length-1 fake -> ~0.
"""

import numpy as np

import concourse.bacc as bacc
import concourse.bass as bass
import concourse.mybir as mybir
import concourse.tile as tile
from concourse.bass_utils import run_bass_kernel_spmd

S = 16384          # segments
N = 8388608        # total elements
NCORES = 8
SPC = S // NCORES  # 2048 segments per core
P = 128            # partitions
K = 1              # segments per partition row (HW: one offset per partition)
SEG_PER_GROUP = P * K          # 128
NG = SPC // SEG_PER_GROUP      # 16 groups per core
NSLOT = NG * K                 # 16 column slots
NEG = -1.0e30

LAST_RESULTS = None  # test harness introspection


def _plan(scope):
    """Host-side metadata from scope only."""
    scope = np.asarray(scope, np.int64).copy()
    gl = np.zeros(S + 1, np.int64)
    gl[1:] = np.cumsum(scope)
    assert gl[-1] == N

    # outlier: the final segment (length patched up by setup_inputs)
    out_len = int(scope[-1])
    assert (scope[:-1] <= 1024).all(), "unexpected long segment outside tail"
    out_core = NCORES - 1
    out_off = int(gl[S - 1] - gl[out_core * SPC])  # local offset in its core
    W8 = max((out_len + P - 1) // P, 1)
    # per-core row lengths for the outlier block ([1,0,...] fake elsewhere)
    lens8 = np.zeros((NCORES, P, 1), np.float32)
    for c in range(NCORES):
        if c == out_core:
            rem = out_len
            for p in range(P):
                lens8[c, p, 0] = min(max(rem, 0), W8)
                rem -= W8
        else:
            lens8[c, 0, 0] = 1.0

    offs_cols = np.zeros((NCORES, P, NSLOT), np.int32)
    lens_cols = np.zeros((NCORES, P, NSLOT), np.float32)
    Fg = [0] * NG
    spans = []
    for c in range(NCORES):
        lens = scope[c * SPC:(c + 1) * SPC].copy()
        base = gl[c * SPC]
        loc = (gl[c * SPC:(c + 1) * SPC] - base).astype(np.int64)
        if c == out_core:
            lens[-1] = 1     # dummy slot replacing the outlier
            loc[-1] = 0
        order = np.argsort(lens, kind="stable")
        sl = lens[order]
        for g in range(NG):
            Fg[g] = max(Fg[g], int(sl[(g + 1) * SEG_PER_GROUP - 1]))
            for j in range(K):
                ranks = order[g * SEG_PER_GROUP + j * P: g * SEG_PER_GROUP + (j + 1) * P]
                offs_cols[c, :, g * K + j] = loc[ranks]
                lens_cols[c, :, g * K + j] = lens[ranks]
        spans.append((int(base), int(gl[(c + 1) * SPC])))
    # bound the dram pad: every gather row reads Fg[g] elements from its offset
    read_end = out_off + P * W8
    for g in range(NG):
        for j in range(K):
            col = g * K + j
            read_end = max(read_end, int(offs_cols[:, :, col].max()) + Fg[g])
    max_len = max(b - a for a, b in spans)
    npc = max(read_end, max_len)
    npc = (npc + 511) // 512 * 512
    return offs_cols, lens_cols, Fg, spans, npc, out_off, W8, lens8


def _build(Fg, npc, out_off, W8):
    f32 = mybir.dt.float32
    i32 = mybir.dt.int32
    Exp = mybir.ActivationFunctionType.Exp
    Ln = mybir.ActivationFunctionType.Ln
    Op = mybir.AluOpType

    nc = bacc.Bacc(num_devices=NCORES)
    t_d = nc.dram_tensor("t", [npc, 1], f32, kind="ExternalInput")
    m_d = nc.dram_tensor("m", [npc, 1], f32, kind="ExternalInput")
    offs_d = nc.dram_tensor("offs", [P, NSLOT], i32, kind="ExternalInput")
    lens_d = nc.dram_tensor("lens", [P, NSLOT], f32, kind="ExternalInput")
    lens8_d = nc.dram_tensor("lens8", [P, 1], f32, kind="ExternalInput")
    out_d = nc.dram_tensor("out", [1, 1], f32, kind="ExternalOutput")
    Fmax = max(max(Fg), W8)

    with tile.TileContext(nc) as tc:
        with tc.tile_pool(name="meta", bufs=1) as meta, \
             tc.tile_pool(name="data", bufs=2) as data, \
             tc.tile_pool(name="acc", bufs=1) as acc, \
             tc.tile_pool(name="psum", bufs=1, space="PSUM") as psp:
            offs_sb = meta.tile([P, NSLOT], i32)
            nc.sync.dma_start(offs_sb[:], offs_d[:])
            lens_sb0 = meta.tile([P, NSLOT], f32)
            nc.sync.dma_start(lens_sb0[:], lens_d[:])
            lens8_sb0 = meta.tile([P, 1], f32)
            nc.sync.dma_start(lens8_sb0[:], lens8_d[:])
            iota_sb0 = meta.tile([P, Fmax], i32)
            nc.gpsimd.iota(iota_sb0[:], pattern=[[1, Fmax]], base=0,
                           channel_multiplier=0)
            # DVE-local staging copies: the per-group TensorScalarPtr mask
            # instructions then depend only on same-engine order (the HW TS
            # encoding has very few sync-wait slots).
            iota_sb = meta.tile([P, Fmax], f32)
            nc.vector.tensor_copy(out=iota_sb[:], in_=iota_sb0[:])
            lens_sb = meta.tile([P, NSLOT], f32)
            nc.vector.tensor_copy(out=lens_sb[:], in_=lens_sb0[:])
            lens8_sb = meta.tile([P, 1], f32)
            nc.vector.tensor_copy(out=lens8_sb[:], in_=lens8_sb0[:])
            ones_sb = meta.tile([P, 1], f32)
            nc.vector.memset(ones_sb[:], 1.0)

            A = acc.tile([P, NSLOT], f32)
            B = acc.tile([P, NSLOT], f32)
            C = acc.tile([P, NSLOT], f32)

            for g in range(NG):
                F = Fg[g]
                cols = K * F
                tt = data.tile([P, cols], f32, tag="tt")
                mt = data.tile([P, cols], f32, tag="mt")
                sc = data.tile([P, cols], f32, tag="sc")
                nc.gpsimd.indirect_dma_start(
                    out=tt[:], out_offset=None, in_=t_d[:],
                    in_offset=bass.IndirectOffsetOnAxis(
                        ap=offs_sb[:, g * K:(g + 1) * K], axis=0))
                nc.gpsimd.indirect_dma_start(
                    out=mt[:], out_offset=None, in_=m_d[:],
                    in_offset=bass.IndirectOffsetOnAxis(
                        ap=offs_sb[:, g * K:(g + 1) * K], axis=0))
                # additive pad mask: 0 where col < len, -1e30 where col >= len
                for j in range(K):
                    col = g * K + j
                    nc.vector.tensor_scalar(
                        out=sc[:, j * F:(j + 1) * F],
                        in0=iota_sb[:, :F],
                        scalar1=lens_sb[:, col:col + 1],
                        scalar2=NEG,
                        op0=Op.is_ge, op1=Op.mult)
                nc.vector.tensor_tensor(out=tt[:], in0=tt[:], in1=sc[:], op=Op.add)
                nc.vector.tensor_tensor(out=mt[:], in0=mt[:], in1=sc[:], op=Op.add)
                for j in range(K):
                    sl = slice(j * F, (j + 1) * F)
                    col = g * K + j
                    # u = exp(t'), A = rowsum(u)
                    nc.scalar.activation(out=sc[:, sl], in_=tt[:, sl], func=Exp,
                                         accum_out=A[:, col:col + 1])
                for j in range(K):
                    sl = slice(j * F, (j + 1) * F)
                    col = g * K + j
                    # w = exp(m'), C = rowsum(w)   (w dead, reuse tt)
                    nc.scalar.activation(out=tt[:, sl], in_=mt[:, sl], func=Exp,
                                         accum_out=C[:, col:col + 1])
                for j in range(K):
                    sl = slice(j * F, (j + 1) * F)
                    col = g * K + j
                    # B = rowsum(u * m')   (v dead, overwrite mt)
                    nc.vector.tensor_tensor(out=mt[:, sl], in0=sc[:, sl],
                                            in1=mt[:, sl], op=Op.mult)
                    nc.vector.reduce_sum(out=B[:, col:col + 1], in_=mt[:, sl],
                                         axis=mybir.AxisListType.X)

            # ---- outlier block: contiguous [128, W8] slab at out_off ----
            abc8 = acc.tile([P, 3], f32)
            t8 = data.tile([P, W8], f32, tag="tt")
            m8 = data.tile([P, W8], f32, tag="mt")
            s8 = data.tile([P, W8], f32, tag="sc")
            t_v = t_d[out_off:out_off + P * W8, :].rearrange(
                "(p f) o -> p (f o)", p=P)
            m_v = m_d[out_off:out_off + P * W8, :].rearrange(
                "(p f) o -> p (f o)", p=P)
            nc.sync.dma_start(t8[:], t_v)
            nc.sync.dma_start(m8[:], m_v)
            nc.vector.tensor_scalar(
                out=s8[:], in0=iota_sb[:, :W8], scalar1=lens8_sb[:, 0:1],
                scalar2=NEG, op0=Op.is_ge, op1=Op.mult)
            nc.vector.tensor_tensor(out=t8[:], in0=t8[:], in1=s8[:], op=Op.add)
            nc.vector.tensor_tensor(out=m8[:], in0=m8[:], in1=s8[:], op=Op.add)
            nc.scalar.activation(out=s8[:], in_=t8[:], func=Exp,
                                 accum_out=abc8[:, 0:1])
            nc.scalar.activation(out=t8[:], in_=m8[:], func=Exp,
                                 accum_out=abc8[:, 2:3])
            nc.vector.tensor_tensor(out=m8[:], in0=s8[:], in1=m8[:], op=Op.mult)
            nc.vector.reduce_sum(out=abc8[:, 1:2], in_=m8[:],
                                 axis=mybir.AxisListType.X)

            # ---- combine: per-slot loss = ln(C) - B/A ----
            ra = acc.tile([P, NSLOT], f32)
            nc.vector.reciprocal(ra[:], A[:])
            nc.vector.tensor_tensor(out=ra[:], in0=B[:], in1=ra[:], op=Op.mult)
            lc = acc.tile([P, NSLOT], f32)
            nc.scalar.activation(out=lc[:], in_=C[:], func=Ln)
            nc.vector.tensor_tensor(out=lc[:], in0=lc[:], in1=ra[:], op=Op.subtract)
            lcol = acc.tile([P, 1], f32)
            nc.vector.reduce_sum(out=lcol[:], in_=lc[:], axis=mybir.AxisListType.X)

            # cross-partition reduction of [loss | A8 B8 C8] in one matmul
            quad = acc.tile([P, 4], f32)
            nc.vector.tensor_copy(out=quad[:, 0:1], in_=lcol[:])
            nc.vector.tensor_copy(out=quad[:, 1:4], in_=abc8[:])
            ps = psp.tile([1, 4], f32)
            nc.tensor.matmul(out=ps[:], lhsT=ones_sb[:], rhs=quad[:],
                             start=True, stop=True)
            tot = acc.tile([1, 4], f32)
            nc.scalar.copy(tot[:], ps[:])
            # loss8 = ln(C8tot) - B8tot/A8tot  (computed on partition 0)
            r8 = acc.tile([1, 3], f32)
            nc.vector.reciprocal(r8[:, 0:1], tot[:, 1:2])
            nc.vector.tensor_tensor(out=r8[:, 1:2], in0=tot[:, 2:3],
                                    in1=r8[:, 0:1], op=Op.mult)
            nc.scalar.activation(out=r8[:, 2:3], in_=tot[:, 3:4], func=Ln)
            res = acc.tile([1, 1], f32)
            nc.vector.tensor_tensor(out=res[:], in0=tot[:, 0:1],
                                    in1=r8[:, 1:2], op=Op.subtract)
            nc.vector.tensor_tensor(out=res[:], in0=res[:],
                                    in1=r8[:, 2:3], op=Op.add)
            nc.sync.dma_start(out_d[:], res[:])
    nc.compile()
    return nc


def _in_maps(means, targets, offs_cols, lens_cols, lens8, spans, npc):
    maps = []
    for c in range(NCORES):
        a, b = spans[c]
        tpad = np.zeros((npc, 1), np.float32)
        tpad[:b - a, 0] = targets[a:b]
        mpad = np.zeros((npc, 1), np.float32)
        mpad[:b - a, 0] = means[a:b]
        maps.append({"t": tpad, "m": mpad,
                     "offs": offs_cols[c], "lens": lens_cols[c],
                     "lens8": lens8[c]})
    return maps


def kernel(means, scope, targets_train):
    global LAST_RESULTS
    means = np.ascontiguousarray(np.asarray(means, dtype=np.float32).reshape(-1))
    targets = np.ascontiguousarray(
        np.asarray(targets_train, dtype=np.float32).reshape(-1))
    offs_cols, lens_cols, Fg, spans, npc, out_off, W8, lens8 = _plan(
        np.asarray(scope))
    nc = _build(Fg, npc, out_off, W8)
    maps = _in_maps(means, targets, offs_cols, lens_cols, lens8, spans, npc)
    res = run_bass_kernel_spmd(nc, maps, core_ids=list(range(NCORES)))
    LAST_RESULTS = res
    total = 0.0
    for c in range(NCORES):
        total += float(res.results[c]["out"][0, 0])
    return np.float32(total / S)


# revision 15
# speedup vs baseline: 1.3812x; 1.3812x over previous
"""ListNet T1 loss on 8 Trainium2 NeuronCores.

Math: for each ragged segment s (contiguous slice of the flat arrays),
    loss_s = log(sum_i exp(m_i)) - (sum_i exp(t_i)*m_i) / (sum_i exp(t_i))
    out    = sum_s loss_s / S
(softmax / log-softmax are shift invariant; inputs are ~N(0,1) so exp() is
safe in fp32 without the max-subtraction trick).

Per core (2048 consecutive segments): sort segments by length, place them as
rows of [128, K*F] SBUF tiles via indirect (gather) DMA with element-granular
offsets, mask the padded row tails to -1e30, then:
  A = sum exp(t)  via ACT Exp + accum_out
  C = sum exp(m)  via ACT Exp + accum_out
  B = sum exp(t)*m via DVE tensor_tensor_reduce
Combine per segment, reduce on-chip to one scalar per core, sum on host.

The final global segment is an outlier (~13.5k elements, vs <=896 for all
others): it is excluded from the sorted grid (its slot becomes a length-1
dummy at offset 0, whose loss is ln(exp(m0)) - m0 ~= 0) and processed by a
dedicated block: one contiguous DMA reshaped to [128, W8], masked, with the
partial A/B/C summed across partitions (ones-matmul) before the log/ratio.
Every core runs that block; non-owner cores get a length-1 fake -> ~0.
"""

import numpy as np

import concourse.bacc as bacc
import concourse.bass as bass
import concourse.mybir as mybir
import concourse.tile as tile
from concourse.bass_utils import run_bass_kernel_spmd

S = 16384          # segments
N = 8388608        # total elements
NCORES = 8
SPC = S // NCORES  # 2048 segments per core
P = 128            # partitions
K = 1              # segments per partition row (HW: one offset per partition)
SEG_PER_GROUP = P * K          # 128
NG = SPC // SEG_PER_GROUP      # 16 groups per core
NSLOT = NG * K                 # 16 column slots
NEG = -1.0e30

LAST_RESULTS = None  # test harness introspection


def _plan(scope):
    """Host-side metadata from scope only."""
    scope = np.asarray(scope, np.int64).copy()
    gl = np.zeros(S + 1, np.int64)
    gl[1:] = np.cumsum(scope)
    assert gl[-1] == N

    # outlier: the final segment (length patched up by setup_inputs)
    out_len = int(scope[-1])
    assert (scope[:-1] <= 1024).all(), "unexpected long segment outside tail"
    out_core = NCORES - 1
    out_off = int(gl[S - 1] - gl[out_core * SPC])  # local offset in its core
    W8 = max((out_len + P - 1) // P, 1)
    # per-core row lengths for the outlier block ([1,0,...] fake elsewhere)
    lens8 = np.zeros((NCORES, P, 1), np.float32)
    for c in range(NCORES):
        if c == out_core:
            rem = out_len
            for p in range(P):
                lens8[c, p, 0] = min(max(rem, 0), W8)
                rem -= W8
        else:
            lens8[c, 0, 0] = 1.0

    offs_cols = np.zeros((NCORES, P, NSLOT), np.int32)
    lens_cols = np.zeros((NCORES, P, NSLOT), np.float32)
    Fg = [0] * NG
    Bg = [1 << 30] * NG   # per-group min length (mask band start)
    spans = []
    for c in range(NCORES):
        lens = scope[c * SPC:(c + 1) * SPC].copy()
        base = gl[c * SPC]
        loc = (gl[c * SPC:(c + 1) * SPC] - base).astype(np.int64)
        if c == out_core:
            lens[-1] = 1     # dummy slot replacing the outlier
            loc[-1] = 0
        order = np.argsort(lens, kind="stable")
        sl = lens[order]
        for g in range(NG):
            Fg[g] = max(Fg[g], int(sl[(g + 1) * SEG_PER_GROUP - 1]))
            Bg[g] = min(Bg[g], int(sl[g * SEG_PER_GROUP]))
            for j in range(K):
                ranks = order[g * SEG_PER_GROUP + j * P: g * SEG_PER_GROUP + (j + 1) * P]
                offs_cols[c, :, g * K + j] = loc[ranks]
                lens_cols[c, :, g * K + j] = lens[ranks]
        spans.append((int(base), int(gl[(c + 1) * SPC])))
    # bound the dram pad: every gather row reads Fg[g] elements from its offset
    read_end = out_off + P * W8
    for g in range(NG):
        for j in range(K):
            col = g * K + j
            read_end = max(read_end, int(offs_cols[:, :, col].max()) + Fg[g])
    max_len = max(b - a for a, b in spans)
    npc = max(read_end, max_len)
    npc = (npc + 511) // 512 * 512
    return offs_cols, lens_cols, Fg, Bg, spans, npc, out_off, W8, lens8


def _build(Fg, Bg, npc, out_off, W8):
    f32 = mybir.dt.float32
    i32 = mybir.dt.int32
    Exp = mybir.ActivationFunctionType.Exp
    Ln = mybir.ActivationFunctionType.Ln
    Op = mybir.AluOpType

    nc = bacc.Bacc(num_devices=NCORES)
    t_d = nc.dram_tensor("t", [npc, 1], f32, kind="ExternalInput")
    m_d = nc.dram_tensor("m", [npc, 1], f32, kind="ExternalInput")
    offs_d = nc.dram_tensor("offs", [P, NSLOT], i32, kind="ExternalInput")
    lens_d = nc.dram_tensor("lens", [P, NSLOT], f32, kind="ExternalInput")
    lens8_d = nc.dram_tensor("lens8", [P, 1], f32, kind="ExternalInput")
    out_d = nc.dram_tensor("out", [1, 1], f32, kind="ExternalOutput")
    Fmax = max(max(Fg), W8)

    with tile.TileContext(nc) as tc:
        with tc.tile_pool(name="meta", bufs=1) as meta, \
             tc.tile_pool(name="data", bufs=4) as data, \
             tc.tile_pool(name="acc", bufs=1) as acc, \
             tc.tile_pool(name="psum", bufs=1, space="PSUM") as psp:
            offs_sb = meta.tile([P, NSLOT], i32)
            nc.sync.dma_start(offs_sb[:], offs_d[:])
            lens_sb0 = meta.tile([P, NSLOT], f32)
            nc.sync.dma_start(lens_sb0[:], lens_d[:])
            lens8_sb0 = meta.tile([P, 1], f32)
            nc.sync.dma_start(lens8_sb0[:], lens8_d[:])
            iota_sb0 = meta.tile([P, Fmax], i32)
            nc.gpsimd.iota(iota_sb0[:], pattern=[[1, Fmax]], base=0,
                           channel_multiplier=0)
            # DVE-local staging copies: the per-group TensorScalarPtr mask
            # instructions then depend only on same-engine order (the HW TS
            # encoding has very few sync-wait slots).
            iota_sb = meta.tile([P, Fmax], f32)
            nc.vector.tensor_copy(out=iota_sb[:], in_=iota_sb0[:])
            lens_sb = meta.tile([P, NSLOT], f32)
            nc.vector.tensor_copy(out=lens_sb[:], in_=lens_sb0[:])
            lens8_sb = meta.tile([P, 1], f32)
            nc.vector.tensor_copy(out=lens8_sb[:], in_=lens8_sb0[:])
            ones_sb = meta.tile([P, 1], f32)
            nc.vector.memset(ones_sb[:], 1.0)

            A = acc.tile([P, NSLOT], f32)
            B = acc.tile([P, NSLOT], f32)
            C = acc.tile([P, NSLOT], f32)

            for g in range(NG):
                F = Fg[g]
                b0 = min(Bg[g], F - 1)   # mask band: columns [b0, F)
                cols = K * F
                tt = data.tile([P, cols], f32, tag="tt")
                mt = data.tile([P, cols], f32, tag="mt")
                sc = data.tile([P, cols], f32, tag="sc")
                nc.gpsimd.indirect_dma_start(
                    out=tt[:], out_offset=None, in_=t_d[:],
                    in_offset=bass.IndirectOffsetOnAxis(
                        ap=offs_sb[:, g * K:(g + 1) * K], axis=0))
                nc.gpsimd.indirect_dma_start(
                    out=mt[:], out_offset=None, in_=m_d[:],
                    in_offset=bass.IndirectOffsetOnAxis(
                        ap=offs_sb[:, g * K:(g + 1) * K], axis=0))
                # additive pad mask: 0 where col < len, -1e30 where col >= len.
                # Only columns >= b0 (the group's min length) can be masked, so
                # the mask build + applies touch just that band.
                for j in range(K):
                    col = g * K + j
                    nc.vector.tensor_scalar(
                        out=sc[:, j * F + b0:(j + 1) * F],
                        in0=iota_sb[:, b0:F],
                        scalar1=lens_sb[:, col:col + 1],
                        scalar2=NEG,
                        op0=Op.is_ge, op1=Op.mult)
                    nc.vector.tensor_tensor(
                        out=tt[:, j * F + b0:(j + 1) * F],
                        in0=tt[:, j * F + b0:(j + 1) * F],
                        in1=sc[:, j * F + b0:(j + 1) * F], op=Op.add)
                    nc.vector.tensor_tensor(
                        out=mt[:, j * F + b0:(j + 1) * F],
                        in0=mt[:, j * F + b0:(j + 1) * F],
                        in1=sc[:, j * F + b0:(j + 1) * F], op=Op.add)
                for j in range(K):
                    sl = slice(j * F, (j + 1) * F)
                    col = g * K + j
                    # u = exp(t'), A = rowsum(u)
                    nc.scalar.activation(out=sc[:, sl], in_=tt[:, sl], func=Exp,
                                         accum_out=A[:, col:col + 1])
                for j in range(K):
                    sl = slice(j * F, (j + 1) * F)
                    col = g * K + j
                    # w = exp(m'), C = rowsum(w)   (w dead, reuse tt)
                    nc.scalar.activation(out=tt[:, sl], in_=mt[:, sl], func=Exp,
                                         accum_out=C[:, col:col + 1])
                for j in range(K):
                    sl = slice(j * F, (j + 1) * F)
                    col = g * K + j
                    # B = rowsum(u * m')   (v dead, overwrite mt)
                    nc.vector.tensor_tensor(out=mt[:, sl], in0=sc[:, sl],
                                            in1=mt[:, sl], op=Op.mult)
                    nc.vector.reduce_sum(out=B[:, col:col + 1], in_=mt[:, sl],
                                         axis=mybir.AxisListType.X)

            # ---- outlier block: contiguous [128, W8] slab at out_off ----
            abc8 = acc.tile([P, 3], f32)
            t8 = data.tile([P, W8], f32, tag="tt")
            m8 = data.tile([P, W8], f32, tag="mt")
            s8 = data.tile([P, W8], f32, tag="sc")
            t_v = t_d[out_off:out_off + P * W8, :].rearrange(
                "(p f) o -> p (f o)", p=P)
            m_v = m_d[out_off:out_off + P * W8, :].rearrange(
                "(p f) o -> p (f o)", p=P)
            nc.sync.dma_start(t8[:], t_v)
            nc.sync.dma_start(m8[:], m_v)
            nc.vector.tensor_scalar(
                out=s8[:], in0=iota_sb[:, :W8], scalar1=lens8_sb[:, 0:1],
                scalar2=NEG, op0=Op.is_ge, op1=Op.mult)
            nc.vector.tensor_tensor(out=t8[:], in0=t8[:], in1=s8[:], op=Op.add)
            nc.vector.tensor_tensor(out=m8[:], in0=m8[:], in1=s8[:], op=Op.add)
            nc.scalar.activation(out=s8[:], in_=t8[:], func=Exp,
                                 accum_out=abc8[:, 0:1])
            nc.scalar.activation(out=t8[:], in_=m8[:], func=Exp,
                                 accum_out=abc8[:, 2:3])
            nc.vector.tensor_tensor(out=m8[:], in0=s8[:], in1=m8[:], op=Op.mult)
            nc.vector.reduce_sum(out=abc8[:, 1:2], in_=m8[:],
                                 axis=mybir.AxisListType.X)

            # ---- combine: per-slot loss = ln(C) - B/A ----
            ra = acc.tile([P, NSLOT], f32)
            nc.vector.reciprocal(ra[:], A[:])
            nc.vector.tensor_tensor(out=ra[:], in0=B[:], in1=ra[:], op=Op.mult)
            lc = acc.tile([P, NSLOT], f32)
            nc.scalar.activation(out=lc[:], in_=C[:], func=Ln)
            nc.vector.tensor_tensor(out=lc[:], in0=lc[:], in1=ra[:], op=Op.subtract)
            lcol = acc.tile([P, 1], f32)
            nc.vector.reduce_sum(out=lcol[:], in_=lc[:], axis=mybir.AxisListType.X)

            # cross-partition reduction of [loss | A8 B8 C8] in one matmul
            quad = acc.tile([P, 4], f32)
            nc.vector.tensor_copy(out=quad[:, 0:1], in_=lcol[:])
            nc.vector.tensor_copy(out=quad[:, 1:4], in_=abc8[:])
            ps = psp.tile([1, 4], f32)
            nc.tensor.matmul(out=ps[:], lhsT=ones_sb[:], rhs=quad[:],
                             start=True, stop=True)
            tot = acc.tile([1, 4], f32)
            nc.scalar.copy(tot[:], ps[:])
            # loss8 = ln(C8tot) - B8tot/A8tot  (computed on partition 0)
            r8 = acc.tile([1, 3], f32)
            nc.vector.reciprocal(r8[:, 0:1], tot[:, 1:2])
            nc.vector.tensor_tensor(out=r8[:, 1:2], in0=tot[:, 2:3],
                                    in1=r8[:, 0:1], op=Op.mult)
            nc.scalar.activation(out=r8[:, 2:3], in_=tot[:, 3:4], func=Ln)
            res = acc.tile([1, 1], f32)
            nc.vector.tensor_tensor(out=res[:], in0=tot[:, 0:1],
                                    in1=r8[:, 1:2], op=Op.subtract)
            nc.vector.tensor_tensor(out=res[:], in0=res[:],
                                    in1=r8[:, 2:3], op=Op.add)
            nc.sync.dma_start(out_d[:], res[:])
    nc.compile()
    return nc


def _in_maps(means, targets, offs_cols, lens_cols, lens8, spans, npc):
    maps = []
    for c in range(NCORES):
        a, b = spans[c]
        tpad = np.zeros((npc, 1), np.float32)
        tpad[:b - a, 0] = targets[a:b]
        mpad = np.zeros((npc, 1), np.float32)
        mpad[:b - a, 0] = means[a:b]
        maps.append({"t": tpad, "m": mpad,
                     "offs": offs_cols[c], "lens": lens_cols[c],
                     "lens8": lens8[c]})
    return maps


def kernel(means, scope, targets_train):
    global LAST_RESULTS
    means = np.ascontiguousarray(np.asarray(means, dtype=np.float32).reshape(-1))
    targets = np.ascontiguousarray(
        np.asarray(targets_train, dtype=np.float32).reshape(-1))
    offs_cols, lens_cols, Fg, Bg, spans, npc, out_off, W8, lens8 = _plan(
        np.asarray(scope))
    nc = _build(Fg, Bg, npc, out_off, W8)
    maps = _in_maps(means, targets, offs_cols, lens_cols, lens8, spans, npc)
    res = run_bass_kernel_spmd(nc, maps, core_ids=list(range(NCORES)))
    LAST_RESULTS = res
    total = 0.0
    for c in range(NCORES):
        total += float(res.results[c]["out"][0, 0])
    return np.float32(total / S)


# revision 18
# speedup vs baseline: 1.4241x; 1.0311x over previous
"""ListNet T1 loss on 8 Trainium2 NeuronCores.

Math: for each ragged segment s (contiguous slice of the flat arrays),
    loss_s = log(sum_i exp(m_i)) - (sum_i exp(t_i)*m_i) / (sum_i exp(t_i))
    out    = sum_s loss_s / S
(softmax / log-softmax are shift invariant; inputs are ~N(0,1) so exp() is
safe in fp32 without the max-subtraction trick).

Per core (2048 consecutive segments): sort segments by length, place them as
rows of [128, K*F] SBUF tiles via indirect (gather) DMA with element-granular
offsets, mask the padded row tails to -1e30, then:
  A = sum exp(t)  via ACT Exp + accum_out
  C = sum exp(m)  via ACT Exp + accum_out
  B = sum exp(t)*m via DVE tensor_tensor_reduce
Combine per segment, reduce on-chip to one scalar per core, sum on host.

The final global segment is an outlier (~13.5k elements, vs <=896 for all
others): it is excluded from the sorted grid (its slot becomes a length-1
dummy at offset 0, whose loss is ln(exp(m0)) - m0 ~= 0) and processed by a
dedicated block: one contiguous DMA reshaped to [128, W8], masked, with the
partial A/B/C summed across partitions (ones-matmul) before the log/ratio.
Every core runs that block; non-owner cores get a length-1 fake -> ~0.
"""

import numpy as np

import concourse.bacc as bacc
import concourse.bass as bass
import concourse.mybir as mybir
import concourse.tile as tile
from concourse.bass_utils import run_bass_kernel_spmd

S = 16384          # segments
N = 8388608        # total elements
NCORES = 8
SPC = S // NCORES  # 2048 segments per core
P = 128            # partitions
K = 1              # segments per partition row (HW: one offset per partition)
SEG_PER_GROUP = P * K          # 128
NG = SPC // SEG_PER_GROUP      # 16 groups per core
NSLOT = NG * K                 # 16 column slots
NEG = -1.0e30

LAST_RESULTS = None  # test harness introspection


def _plan(scope):
    """Host-side metadata from scope only."""
    scope = np.asarray(scope, np.int64).copy()
    gl = np.zeros(S + 1, np.int64)
    gl[1:] = np.cumsum(scope)
    assert gl[-1] == N

    # outlier: the final segment (length patched up by setup_inputs)
    out_len = int(scope[-1])
    assert (scope[:-1] <= 1024).all(), "unexpected long segment outside tail"
    out_core = NCORES - 1
    out_off = int(gl[S - 1] - gl[out_core * SPC])  # local offset in its core
    W8 = max((out_len + P - 1) // P, 1)
    # per-core row lengths for the outlier block ([1,0,...] fake elsewhere)
    lens8 = np.zeros((NCORES, P, 1), np.float32)
    for c in range(NCORES):
        if c == out_core:
            rem = out_len
            for p in range(P):
                lens8[c, p, 0] = min(max(rem, 0), W8)
                rem -= W8
        else:
            lens8[c, 0, 0] = 1.0

    offs_cols = np.zeros((NCORES, P, NSLOT), np.int32)
    lens_cols = np.zeros((NCORES, P, NSLOT), np.float32)
    Fg = [0] * NG
    Bg = [1 << 30] * NG   # per-group min length (mask band start)
    spans = []
    for c in range(NCORES):
        lens = scope[c * SPC:(c + 1) * SPC].copy()
        base = gl[c * SPC]
        loc = (gl[c * SPC:(c + 1) * SPC] - base).astype(np.int64)
        if c == out_core:
            lens[-1] = 1     # dummy slot replacing the outlier
            loc[-1] = 0
        order = np.argsort(lens, kind="stable")
        sl = lens[order]
        for g in range(NG):
            Fg[g] = max(Fg[g], int(sl[(g + 1) * SEG_PER_GROUP - 1]))
            Bg[g] = min(Bg[g], int(sl[g * SEG_PER_GROUP]))
            for j in range(K):
                ranks = order[g * SEG_PER_GROUP + j * P: g * SEG_PER_GROUP + (j + 1) * P]
                offs_cols[c, :, g * K + j] = loc[ranks]
                lens_cols[c, :, g * K + j] = lens[ranks]
        spans.append((int(base), int(gl[(c + 1) * SPC])))
    # bound the dram pad: every gather row reads Fg[g] elements from its offset
    read_end = out_off + P * W8
    for g in range(NG):
        for j in range(K):
            col = g * K + j
            read_end = max(read_end, int(offs_cols[:, :, col].max()) + Fg[g])
    max_len = max(b - a for a, b in spans)
    npc = max(read_end, max_len)
    npc = (npc + 511) // 512 * 512
    return offs_cols, lens_cols, Fg, Bg, spans, npc, out_off, W8, lens8


def _build(Fg, Bg, npc, out_off, W8):
    f32 = mybir.dt.float32
    i32 = mybir.dt.int32
    Exp = mybir.ActivationFunctionType.Exp
    Ln = mybir.ActivationFunctionType.Ln
    Op = mybir.AluOpType

    nc = bacc.Bacc(num_devices=NCORES)
    t_d = nc.dram_tensor("t", [npc, 1], f32, kind="ExternalInput")
    m_d = nc.dram_tensor("m", [npc, 1], f32, kind="ExternalInput")
    offs_d = nc.dram_tensor("offs", [P, NSLOT], i32, kind="ExternalInput")
    lens_d = nc.dram_tensor("lens", [P, NSLOT], f32, kind="ExternalInput")
    lens8_d = nc.dram_tensor("lens8", [P, 1], f32, kind="ExternalInput")
    out_d = nc.dram_tensor("out", [1, 1], f32, kind="ExternalOutput")
    Fmax = max(max(Fg), W8)

    with tile.TileContext(nc) as tc:
        with tc.tile_pool(name="meta", bufs=1) as meta, \
             tc.tile_pool(name="data", bufs=8) as data, \
             tc.tile_pool(name="acc", bufs=1) as acc, \
             tc.tile_pool(name="psum", bufs=1, space="PSUM") as psp:
            offs_sb = meta.tile([P, NSLOT], i32)
            nc.sync.dma_start(offs_sb[:], offs_d[:])
            lens_sb0 = meta.tile([P, NSLOT], f32)
            nc.sync.dma_start(lens_sb0[:], lens_d[:])
            lens8_sb0 = meta.tile([P, 1], f32)
            nc.sync.dma_start(lens8_sb0[:], lens8_d[:])
            iota_sb0 = meta.tile([P, Fmax], i32)
            nc.gpsimd.iota(iota_sb0[:], pattern=[[1, Fmax]], base=0,
                           channel_multiplier=0)
            # DVE-local staging copies: the per-group TensorScalarPtr mask
            # instructions then depend only on same-engine order (the HW TS
            # encoding has very few sync-wait slots).
            iota_sb = meta.tile([P, Fmax], f32)
            nc.vector.tensor_copy(out=iota_sb[:], in_=iota_sb0[:])
            lens_sb = meta.tile([P, NSLOT], f32)
            nc.vector.tensor_copy(out=lens_sb[:], in_=lens_sb0[:])
            lens8_sb = meta.tile([P, 1], f32)
            nc.vector.tensor_copy(out=lens8_sb[:], in_=lens8_sb0[:])
            ones_sb = meta.tile([P, 1], f32)
            nc.vector.memset(ones_sb[:], 1.0)

            A = acc.tile([P, NSLOT], f32)
            B = acc.tile([P, NSLOT], f32)
            C = acc.tile([P, NSLOT], f32)

            for g in range(NG):
                F = Fg[g]
                b0 = min(Bg[g], F - 1)   # mask band: columns [b0, F)
                cols = K * F
                tt = data.tile([P, cols], f32, tag="tt")
                mt = data.tile([P, cols], f32, tag="mt")
                sc = data.tile([P, cols], f32, tag="sc")
                nc.gpsimd.indirect_dma_start(
                    out=tt[:], out_offset=None, in_=t_d[:],
                    in_offset=bass.IndirectOffsetOnAxis(
                        ap=offs_sb[:, g * K:(g + 1) * K], axis=0))
                nc.gpsimd.indirect_dma_start(
                    out=mt[:], out_offset=None, in_=m_d[:],
                    in_offset=bass.IndirectOffsetOnAxis(
                        ap=offs_sb[:, g * K:(g + 1) * K], axis=0))
                # additive pad mask: 0 where col < len, -1e30 where col >= len.
                # Only columns >= b0 (the group's min length) can be masked, so
                # the mask build + applies touch just that band.
                for j in range(K):
                    col = g * K + j
                    nc.vector.tensor_scalar(
                        out=sc[:, j * F + b0:(j + 1) * F],
                        in0=iota_sb[:, b0:F],
                        scalar1=lens_sb[:, col:col + 1],
                        scalar2=NEG,
                        op0=Op.is_ge, op1=Op.mult)
                    nc.vector.tensor_tensor(
                        out=tt[:, j * F + b0:(j + 1) * F],
                        in0=tt[:, j * F + b0:(j + 1) * F],
                        in1=sc[:, j * F + b0:(j + 1) * F], op=Op.add)
                    nc.vector.tensor_tensor(
                        out=mt[:, j * F + b0:(j + 1) * F],
                        in0=mt[:, j * F + b0:(j + 1) * F],
                        in1=sc[:, j * F + b0:(j + 1) * F], op=Op.add)
                for j in range(K):
                    sl = slice(j * F, (j + 1) * F)
                    col = g * K + j
                    # u = exp(t'), A = rowsum(u)
                    nc.scalar.activation(out=sc[:, sl], in_=tt[:, sl], func=Exp,
                                         accum_out=A[:, col:col + 1])
                for j in range(K):
                    sl = slice(j * F, (j + 1) * F)
                    col = g * K + j
                    # w = exp(m'), C = rowsum(w)   (w dead, reuse tt)
                    nc.scalar.activation(out=tt[:, sl], in_=mt[:, sl], func=Exp,
                                         accum_out=C[:, col:col + 1])
                for j in range(K):
                    sl = slice(j * F, (j + 1) * F)
                    col = g * K + j
                    # B = rowsum(u * m')   (v dead, overwrite mt)
                    nc.vector.scalar_tensor_tensor(
                        out=mt[:, sl], in0=sc[:, sl], scalar=1.0,
                        in1=mt[:, sl], op0=Op.mult, op1=Op.mult,
                        accum_out=B[:, col:col + 1])

            # ---- outlier block: contiguous [128, W8] slab at out_off ----
            abc8 = acc.tile([P, 3], f32)
            t8 = data.tile([P, W8], f32, tag="tt")
            m8 = data.tile([P, W8], f32, tag="mt")
            s8 = data.tile([P, W8], f32, tag="sc")
            t_v = t_d[out_off:out_off + P * W8, :].rearrange(
                "(p f) o -> p (f o)", p=P)
            m_v = m_d[out_off:out_off + P * W8, :].rearrange(
                "(p f) o -> p (f o)", p=P)
            nc.sync.dma_start(t8[:], t_v)
            nc.sync.dma_start(m8[:], m_v)
            nc.vector.tensor_scalar(
                out=s8[:], in0=iota_sb[:, :W8], scalar1=lens8_sb[:, 0:1],
                scalar2=NEG, op0=Op.is_ge, op1=Op.mult)
            nc.vector.tensor_tensor(out=t8[:], in0=t8[:], in1=s8[:], op=Op.add)
            nc.vector.tensor_tensor(out=m8[:], in0=m8[:], in1=s8[:], op=Op.add)
            nc.scalar.activation(out=s8[:], in_=t8[:], func=Exp,
                                 accum_out=abc8[:, 0:1])
            nc.scalar.activation(out=t8[:], in_=m8[:], func=Exp,
                                 accum_out=abc8[:, 2:3])
            nc.vector.scalar_tensor_tensor(
                out=m8[:], in0=s8[:], scalar=1.0, in1=m8[:],
                op0=Op.mult, op1=Op.mult, accum_out=abc8[:, 1:2])

            # ---- combine: per-slot loss = ln(C) - B/A ----
            ra = acc.tile([P, NSLOT], f32)
            nc.vector.reciprocal(ra[:], A[:])
            nc.vector.tensor_tensor(out=ra[:], in0=B[:], in1=ra[:], op=Op.mult)
            lc = acc.tile([P, NSLOT], f32)
            nc.scalar.activation(out=lc[:], in_=C[:], func=Ln)
            nc.vector.tensor_tensor(out=lc[:], in0=lc[:], in1=ra[:], op=Op.subtract)
            lcol = acc.tile([P, 1], f32)
            nc.vector.reduce_sum(out=lcol[:], in_=lc[:], axis=mybir.AxisListType.X)

            # cross-partition reduction of [loss | A8 B8 C8] in one matmul
            quad = acc.tile([P, 4], f32)
            nc.vector.tensor_copy(out=quad[:, 0:1], in_=lcol[:])
            nc.vector.tensor_copy(out=quad[:, 1:4], in_=abc8[:])
            ps = psp.tile([1, 4], f32)
            nc.tensor.matmul(out=ps[:], lhsT=ones_sb[:], rhs=quad[:],
                             start=True, stop=True)
            tot = acc.tile([1, 4], f32)
            nc.scalar.copy(tot[:], ps[:])
            # loss8 = ln(C8tot) - B8tot/A8tot  (computed on partition 0)
            r8 = acc.tile([1, 3], f32)
            nc.vector.reciprocal(r8[:, 0:1], tot[:, 1:2])
            nc.vector.tensor_tensor(out=r8[:, 1:2], in0=tot[:, 2:3],
                                    in1=r8[:, 0:1], op=Op.mult)
            nc.scalar.activation(out=r8[:, 2:3], in_=tot[:, 3:4], func=Ln)
            res = acc.tile([1, 1], f32)
            nc.vector.tensor_tensor(out=res[:], in0=tot[:, 0:1],
                                    in1=r8[:, 1:2], op=Op.subtract)
            nc.vector.tensor_tensor(out=res[:], in0=res[:],
                                    in1=r8[:, 2:3], op=Op.add)
            nc.sync.dma_start(out_d[:], res[:])
    nc.compile()
    return nc


def _in_maps(means, targets, offs_cols, lens_cols, lens8, spans, npc):
    maps = []
    for c in range(NCORES):
        a, b = spans[c]
        tpad = np.zeros((npc, 1), np.float32)
        tpad[:b - a, 0] = targets[a:b]
        mpad = np.zeros((npc, 1), np.float32)
        mpad[:b - a, 0] = means[a:b]
        maps.append({"t": tpad, "m": mpad,
                     "offs": offs_cols[c], "lens": lens_cols[c],
                     "lens8": lens8[c]})
    return maps


def kernel(means, scope, targets_train):
    global LAST_RESULTS
    means = np.ascontiguousarray(np.asarray(means, dtype=np.float32).reshape(-1))
    targets = np.ascontiguousarray(
        np.asarray(targets_train, dtype=np.float32).reshape(-1))
    offs_cols, lens_cols, Fg, Bg, spans, npc, out_off, W8, lens8 = _plan(
        np.asarray(scope))
    nc = _build(Fg, Bg, npc, out_off, W8)
    maps = _in_maps(means, targets, offs_cols, lens_cols, lens8, spans, npc)
    res = run_bass_kernel_spmd(nc, maps, core_ids=list(range(NCORES)))
    LAST_RESULTS = res
    total = 0.0
    for c in range(NCORES):
        total += float(res.results[c]["out"][0, 0])
    return np.float32(total / S)
